# revision 8
# baseline (speedup 1.0000x reference)
"""BitLinear158 forward on 8 Trainium2 NeuronCores.

y = x @ quantize(W).T where quantize is the absmean ternary quantizer:
    gamma = mean(|W|) + 1e-6 ; qw = sign(W) * min(round(|W/gamma|), 1)

Strategy (tensor parallel over out_features, x replicated):
  - host: replicate the reference quantizer bit-exactly (jax on CPU) and ship
    the ternary weight shard directly as fp8e4 ({-1,0,1} are exact in fp8).
  - host: round x once to fp8e4 (TRN FP8_EXP4 == ml_dtypes.float8_e4m3 for
    our range) and pre-arrange both operands into the exact SBUF layouts the
    kernel wants, so every DMA is a long contiguous per-partition burst.
  - each core: one fp8 matmul pass in DoubleRow perf mode (contraction 256
    per pass, 2x PE throughput) accumulating f32 in PSUM.
  - fallback MODE="bf16": one bf16 pass (no perf mode), ~2x slower but
    ~10x more accurate; used only if the fp8 path misses the error gate.
"""

import numpy as np
import ml_dtypes

import concourse.bass as bass
import concourse.bacc as bacc
import concourse.mybir as mybir
import concourse.tile as tile
from concourse import bass_utils

# Problem shapes (hardcoded per contract).
B, S, D_IN, D_OUT = 4, 2048, 4096, 16384
N_CORES = 8
O_PER = D_OUT // N_CORES          # 2048 out-features per core
T_TOK = B * S                     # 8192 tokens
KS = D_IN // 128                  # 32 k-slabs of 128
TT = T_TOK // 128                 # 64 token tiles
EPS = 1e-6

MODE = "fp8dr"                    # "fp8dr" | "bf16"

# Set by test harness to capture profiling info; leave False for grading.
TRACE = False
TMPDIR = None
LAST_RESULTS = None


def _quantize_ref(weight: np.ndarray) -> np.ndarray:
    """Bit-exact replication of reference.absmean_quantize (eager jax on the
    default backend, matching how the reference executes); numpy fallback."""
    try:
        import jax
        import jax.numpy as jnp
        from contextlib import nullcontext

        try:
            ctx = jax.default_device(jax.devices("cpu")[0])
        except Exception:
            ctx = nullcontext()
        with ctx:
            gamma = jnp.abs(weight).mean() + EPS
            ws = weight / gamma
            qw = jnp.sign(ws) * jnp.minimum(jnp.round(jnp.abs(ws)), 1.0)
            return np.asarray(qw)
    except Exception:
        gamma = np.float32(np.abs(weight).mean(dtype=np.float64)) + np.float32(EPS)
        ws = (weight / gamma).astype(np.float32)
        return (np.sign(ws) * np.minimum(np.round(np.abs(ws)), np.float32(1.0))
                ).astype(np.float32)


def build_program() -> bass.Bass:
    """Emit the per-core Bass/Tile program.

    DRAM I/O (per core), fp8 DoubleRow mode:
      xd [128, TT*KS*128] fp8e4 -- x pre-arranged [p][tt][k][ti]; per token
                                   tile the DMA is 4 KiB contiguous/partition
      wd [128, KS*O_PER]  fp8e4 -- ternary W.T shard, [p][k][o]
      y  [TT*128, O_PER]  f32   -- this core's output slice
    """
    fp8 = MODE == "fp8dr"
    in_dt = mybir.dt.float8e4 if fp8 else mybir.dt.bfloat16
    NCHUNK = O_PER // 512

    nc = bacc.Bacc("TRN2", target_bir_lowering=False, debug=False)
    xd = nc.dram_tensor("xd", [128, TT * KS * 128], in_dt, kind="ExternalInput")
    wd = nc.dram_tensor("wd", [128, KS * O_PER], in_dt, kind="ExternalInput")
    y = nc.dram_tensor("y", [TT * 128, O_PER], mybir.dt.float32,
                       kind="ExternalOutput")

    xr = xd.ap().rearrange("p (tt k t) -> p tt k t", tt=TT, k=KS)
    wr = wd.ap().rearrange("p (k o) -> p k o", k=KS)

    with tile.TileContext(nc) as tc:
        with (
            tc.tile_pool(name="qw", bufs=1) as qw_pool,
            tc.tile_pool(name="xin", bufs=3) as xin_pool,
            tc.tile_pool(name="warm", bufs=1) as warm_pool,
            tc.tile_pool(name="outs", bufs=2) as out_pool,
            tc.tile_pool(name="psum", bufs=2, space="PSUM") as psum_pool,
        ):
            def x_load(t):
                xt = xin_pool.tile([128, KS, 128], in_dt, name="xt", tag="xt")
                nc.gpsimd.dma_start(out=xt, in_=xr[:, t, :, :])
                return xt

            xcur = x_load(0)

            # ---- resident ternary weight shard, chunked so the first
            # ktile's matmuls can start before the whole shard lands;
            # alternate DMA queues to double the early supply rate ----
            qw = qw_pool.tile([128, KS, O_PER], in_dt)
            for k in range(KS):
                eng = nc.sync if k % 2 == 0 else nc.gpsimd
                eng.dma_start(out=qw[:, k, :], in_=wr[:, k, :])

            # ---- HAM warmup: dummy matmuls on memset tiles run during the
            # initial DMA wait so the real stream starts at the 2.4 GHz
            # clock instead of paying ~24 cold matmuls ----
            if fp8:
                wx = warm_pool.tile([128, 2, 128], in_dt)
                ww = warm_pool.tile([128, 2, 512], in_dt)
                nc.vector.memset(wx, 0)
                nc.vector.memset(ww, 0)
                psd = psum_pool.tile([128, 512], mybir.dt.float32,
                                     name="ps0", tag="ps0")
                for _ in range(20):
                    nc.tensor.matmul(
                        psd, wx, ww, start=True, stop=True,
                        perf_mode=mybir.MatmulPerfMode.DoubleRow,
                        skip_group_check=True,
                    )

            # ---- main loop over 128-token tiles ----
            for t in range(TT):
                xt = xcur
                if t + 1 < TT:
                    xcur = x_load(t + 1)

                ot = out_pool.tile([128, O_PER], mybir.dt.float32)
                pss = [psum_pool.tile([128, 512], mybir.dt.float32,
                                      name=f"ps{c}", tag=f"ps{c}")
                       for c in range(NCHUNK)]
                if fp8:
                    for kk in range(KS // 2):
                        for c in range(NCHUNK):
                            nc.tensor.matmul(
                                pss[c],
                                xt[:, 2 * kk:2 * kk + 2, :],
                                qw[:, 2 * kk:2 * kk + 2,
                                   c * 512:(c + 1) * 512],
                                start=(kk == 0),
                                stop=(kk == KS // 2 - 1),
                                perf_mode=mybir.MatmulPerfMode.DoubleRow,
                            )
                else:
                    for k in range(KS):
                        for c in range(NCHUNK):
                            nc.tensor.matmul(
                                pss[c],
                                xt[:, k, :],
                                qw[:, k, c * 512:(c + 1) * 512],
                                start=(k == 0),
                                stop=(k == KS - 1),
                            )
                # copies alternate scalar/vector so the last tile's four
                # copies pair up instead of serializing; per-chunk output
                # DMAs shrink the final exposed transfer 4x
                for c in range(NCHUNK):
                    if c % 2 == 0:
                        nc.scalar.copy(out=ot[:, c * 512:(c + 1) * 512],
                                       in_=pss[c])
                    else:
                        nc.vector.tensor_copy(
                            out=ot[:, c * 512:(c + 1) * 512], in_=pss[c])
                    nc.scalar.dma_start(
                        out=y.ap()[t * 128:(t + 1) * 128,
                                   c * 512:(c + 1) * 512],
                        in_=ot[:, c * 512:(c + 1) * 512],
                    )
    nc.compile()
    return nc


def _prep_inputs(x: np.ndarray, weight: np.ndarray):
    """Host-side layout prep.

    xd[p, tt, k, ti] = x[tt*128+ti, k*128+p]   (tokens flattened)
    wd[p, k, o]      = qw[o_global, k*128+p]   per-core o slice
    """
    np8 = ml_dtypes.float8_e4m3 if MODE == "fp8dr" else ml_dtypes.bfloat16
    qw = _quantize_ref(weight)

    xq = x.reshape(T_TOK, D_IN).astype(np8)          # [T, K] rounded once
    # -> [p, tt, k, ti]
    xd = np.ascontiguousarray(
        xq.reshape(TT, 128, KS, 128).transpose(3, 0, 2, 1)
    ).reshape(128, TT * KS * 128)

    wq = qw.astype(np8)                               # [D_OUT, K] exact
    # -> per-core [p, k, o]
    wds = []
    for c in range(N_CORES):
        wc = wq[c * O_PER:(c + 1) * O_PER]            # [O_PER, K]
        wd = np.ascontiguousarray(
            wc.reshape(O_PER, KS, 128).transpose(2, 1, 0)
        ).reshape(128, KS * O_PER)
        wds.append(wd)
    return xd, wds


def kernel(x: np.ndarray, weight: np.ndarray) -> np.ndarray:
    global LAST_RESULTS
    assert x.shape == (B, S, D_IN) and weight.shape == (D_OUT, D_IN)

    xd, wds = _prep_inputs(x, weight)
    nc = build_program()
    in_maps = [{"xd": xd, "wd": wds[c]} for c in range(N_CORES)]
    res = bass_utils.run_bass_kernel_spmd(
        nc, in_maps, list(range(N_CORES)), trace=TRACE, tmpdir=TMPDIR,
    )
    LAST_RESULTS = res
    yout = np.concatenate([res.results[c]["y"] for c in range(N_CORES)],
                          axis=1)
    return np.ascontiguousarray(
        yout.reshape(B, S, D_OUT).astype(np.float32, copy=False))


# revision 11
# speedup vs baseline: 1.0079x; 1.0079x over previous
"""BitLinear158 forward on 8 Trainium2 NeuronCores.

y = x @ quantize(W).T where quantize is the absmean ternary quantizer:
    gamma = mean(|W|) + 1e-6 ; qw = sign(W) * min(round(|W/gamma|), 1)

Strategy (tensor parallel over out_features, x replicated):
  - host: replicate the reference quantizer bit-exactly (jax on CPU) and ship
    the ternary weight shard directly as fp8e4 ({-1,0,1} are exact in fp8).
  - host: round x once to fp8e4 (TRN FP8_EXP4 == ml_dtypes.float8_e4m3 for
    our range) and pre-arrange both operands into the exact SBUF layouts the
    kernel wants, so every DMA is a long contiguous per-partition burst.
  - each core: one fp8 matmul pass in DoubleRow perf mode (contraction 256
    per pass, 2x PE throughput) accumulating f32 in PSUM.
  - fallback MODE="bf16": one bf16 pass (no perf mode), ~2x slower but
    ~10x more accurate; used only if the fp8 path misses the error gate.
"""

import numpy as np
import ml_dtypes

import concourse.bass as bass
import concourse.bacc as bacc
import concourse.mybir as mybir
import concourse.tile as tile
from concourse import bass_utils

# Problem shapes (hardcoded per contract).
B, S, D_IN, D_OUT = 4, 2048, 4096, 16384
N_CORES = 8
O_PER = D_OUT // N_CORES          # 2048 out-features per core
T_TOK = B * S                     # 8192 tokens
KS = D_IN // 128                  # 32 k-slabs of 128
TT = T_TOK // 128                 # 64 token tiles
EPS = 1e-6

MODE = "fp8dr"                    # "fp8dr" | "bf16"

# Set by test harness to capture profiling info; leave False for grading.
TRACE = False
TMPDIR = None
LAST_RESULTS = None


def _quantize_ref(weight: np.ndarray) -> np.ndarray:
    """Bit-exact replication of reference.absmean_quantize (eager jax on the
    default backend, matching how the reference executes); numpy fallback."""
    try:
        import jax
        import jax.numpy as jnp
        from contextlib import nullcontext

        try:
            ctx = jax.default_device(jax.devices("cpu")[0])
        except Exception:
            ctx = nullcontext()
        with ctx:
            gamma = jnp.abs(weight).mean() + EPS
            ws = weight / gamma
            qw = jnp.sign(ws) * jnp.minimum(jnp.round(jnp.abs(ws)), 1.0)
            return np.asarray(qw)
    except Exception:
        gamma = np.float32(np.abs(weight).mean(dtype=np.float64)) + np.float32(EPS)
        ws = (weight / gamma).astype(np.float32)
        return (np.sign(ws) * np.minimum(np.round(np.abs(ws)), np.float32(1.0))
                ).astype(np.float32)


def build_program() -> bass.Bass:
    """Emit the per-core Bass/Tile program.

    DRAM I/O (per core), fp8 DoubleRow mode:
      xd [128, TT*KS*128] fp8e4 -- x pre-arranged [p][tt][k][ti]; per token
                                   tile the DMA is 4 KiB contiguous/partition
      wd [128, KS*O_PER]  fp8e4 -- ternary W.T shard, [p][k][o]
      y  [TT*128, O_PER]  f32   -- this core's output slice
    """
    fp8 = MODE == "fp8dr"
    in_dt = mybir.dt.float8e4 if fp8 else mybir.dt.bfloat16
    NCHUNK = O_PER // 512

    nc = bacc.Bacc("TRN2", target_bir_lowering=False, debug=False)
    xd = nc.dram_tensor("xd", [128, TT * KS * 128], in_dt, kind="ExternalInput")
    wd = nc.dram_tensor("wd", [128, KS * O_PER], in_dt, kind="ExternalInput")
    y = nc.dram_tensor("y", [TT * 128, O_PER], mybir.dt.float32,
                       kind="ExternalOutput")

    xr = xd.ap().rearrange("p (tt k t) -> p tt k t", tt=TT, k=KS)
    wr = wd.ap().rearrange("p (k o) -> p k o", k=KS)

    with tile.TileContext(nc) as tc:
        with (
            tc.tile_pool(name="qw", bufs=1) as qw_pool,
            tc.tile_pool(name="xin", bufs=3) as xin_pool,
            tc.tile_pool(name="warm", bufs=1) as warm_pool,
            tc.tile_pool(name="outs", bufs=2) as out_pool,
            tc.tile_pool(name="psum", bufs=2, space="PSUM") as psum_pool,
        ):
            def x_load(t):
                xt = xin_pool.tile([128, KS, 128], in_dt, name="xt", tag="xt")
                nc.gpsimd.dma_start(out=xt, in_=xr[:, t, :, :])
                return xt

            xcur = x_load(0)

            # ---- resident ternary weight shard, chunked so the first
            # ktile's matmuls can start before the whole shard lands;
            # alternate DMA queues to double the early supply rate ----
            qw = qw_pool.tile([128, KS, O_PER], in_dt)
            for k in range(KS):
                eng = nc.sync if k % 2 == 0 else nc.scalar
                eng.dma_start(out=qw[:, k, :], in_=wr[:, k, :])

            # ---- HAM warmup: dummy matmuls on memset tiles run during the
            # initial DMA wait so the real stream starts at the 2.4 GHz
            # clock instead of paying ~24 cold matmuls ----
            if fp8:
                wx = warm_pool.tile([128, 2, 128], in_dt)
                ww = warm_pool.tile([128, 2, 512], in_dt)
                nc.vector.memset(wx, 0)
                nc.vector.memset(ww, 0)
                psd = psum_pool.tile([128, 512], mybir.dt.float32,
                                     name="ps0", tag="ps0")
                for _ in range(20):
                    nc.tensor.matmul(
                        psd, wx, ww, start=True, stop=True,
                        perf_mode=mybir.MatmulPerfMode.DoubleRow,
                        skip_group_check=True,
                    )

            # ---- main loop over 128-token tiles ----
            for t in range(TT):
                xt = xcur
                if t + 1 < TT:
                    xcur = x_load(t + 1)

                ot = out_pool.tile([128, O_PER], mybir.dt.float32)
                pss = [psum_pool.tile([128, 512], mybir.dt.float32,
                                      name=f"ps{c}", tag=f"ps{c}")
                       for c in range(NCHUNK)]
                if fp8:
                    for kk in range(KS // 2):
                        for c in range(NCHUNK):
                            nc.tensor.matmul(
                                pss[c],
                                xt[:, 2 * kk:2 * kk + 2, :],
                                qw[:, 2 * kk:2 * kk + 2,
                                   c * 512:(c + 1) * 512],
                                start=(kk == 0),
                                stop=(kk == KS // 2 - 1),
                                perf_mode=mybir.MatmulPerfMode.DoubleRow,
                            )
                else:
                    for k in range(KS):
                        for c in range(NCHUNK):
                            nc.tensor.matmul(
                                pss[c],
                                xt[:, k, :],
                                qw[:, k, c * 512:(c + 1) * 512],
                                start=(k == 0),
                                stop=(k == KS - 1),
                            )
                # copies alternate scalar/vector so the last tile's four
                # copies pair up instead of serializing. One output DMA per
                # tile (extra DMAs exhaust the semaphore pool and chain the
                # weight-supply queues); only the final tile splits its DMA
                # per chunk to shrink the exposed tail transfer.
                for c in range(NCHUNK):
                    if c % 2 == 0:
                        nc.scalar.copy(out=ot[:, c * 512:(c + 1) * 512],
                                       in_=pss[c])
                    else:
                        nc.vector.tensor_copy(
                            out=ot[:, c * 512:(c + 1) * 512], in_=pss[c])
                    if t == TT - 1:
                        nc.scalar.dma_start(
                            out=y.ap()[t * 128:(t + 1) * 128,
                                       c * 512:(c + 1) * 512],
                            in_=ot[:, c * 512:(c + 1) * 512],
                        )
                if t < TT - 1:
                    nc.scalar.dma_start(
                        out=y.ap()[t * 128:(t + 1) * 128, :], in_=ot,
                    )
    nc.compile()
    return nc


def _prep_inputs(x: np.ndarray, weight: np.ndarray):
    """Host-side layout prep.

    xd[p, tt, k, ti] = x[tt*128+ti, k*128+p]   (tokens flattened)
    wd[p, k, o]      = qw[o_global, k*128+p]   per-core o slice
    """
    np8 = ml_dtypes.float8_e4m3 if MODE == "fp8dr" else ml_dtypes.bfloat16
    qw = _quantize_ref(weight)

    xq = x.reshape(T_TOK, D_IN).astype(np8)          # [T, K] rounded once
    # -> [p, tt, k, ti]
    xd = np.ascontiguousarray(
        xq.reshape(TT, 128, KS, 128).transpose(3, 0, 2, 1)
    ).reshape(128, TT * KS * 128)

    wq = qw.astype(np8)                               # [D_OUT, K] exact
    # -> per-core [p, k, o]
    wds = []
    for c in range(N_CORES):
        wc = wq[c * O_PER:(c + 1) * O_PER]            # [O_PER, K]
        wd = np.ascontiguousarray(
            wc.reshape(O_PER, KS, 128).transpose(2, 1, 0)
        ).reshape(128, KS * O_PER)
        wds.append(wd)
    return xd, wds


def kernel(x: np.ndarray, weight: np.ndarray) -> np.ndarray:
    global LAST_RESULTS
    x = np.asarray(x, dtype=np.float32)
    weight = np.asarray(weight, dtype=np.float32)
    assert x.shape == (B, S, D_IN) and weight.shape == (D_OUT, D_IN)

    xd, wds = _prep_inputs(x, weight)
    nc = build_program()
    in_maps = [{"xd": xd, "wd": wds[c]} for c in range(N_CORES)]
    res = bass_utils.run_bass_kernel_spmd(
        nc, in_maps, list(range(N_CORES)), trace=TRACE, tmpdir=TMPDIR,
    )
    LAST_RESULTS = res
    yout = np.concatenate([res.results[c]["y"] for c in range(N_CORES)],
                          axis=1)
    return np.ascontiguousarray(
        yout.reshape(B, S, D_OUT).astype(np.float32, copy=False))
